# revision 23
# baseline (speedup 1.0000x reference)
"""Trainium2 Bass kernel for nn_Attention_3934190044008.

Multi-head attention with additive bias and sigmoid gating:
  q = (q_x @ w_q) / 8, k = kv_x @ w_k, v = kv_x @ w_v   (8 heads x 64)
  a = softmax(q k^T + bias);  o = a @ v
  o = o * sigmoid(q_x @ w_g + b_g);  out = o @ w_o + b_o

Sharding: 16 (batch, head) pairs over 8 cores -> each core owns one batch
element and 2 heads, produces a partial [2048, 256] output contribution
(o_slice @ w_o rows); host sums the 4 partials per batch and adds b_o.

Device-side layout is "feature on partitions" (transposed): scores are
computed as S^T [k, q] so the softmax denominator rides the AV matmul via a
ones-column appended to V, and softmax-over-k never needs a partition-axis
reduction. All transposes are done on the host (numpy).

v5: q-block-outer loop, both heads interleaved per k-tile. The two heads'
64-contract QK matmuls sit on partitions 0-63 / 64-127 so they land in
disjoint PE row groups and run CONCURRENTLY (hw-verified 2x: 114 ns/MM
packed vs 227 serial); same packing for the per-head output projections.
All matmuls are f32r (tf32-class, 1 cycle/row); the additive bias ships
as bf16 (halves the 33.5 MB/core stream) and is added on the PE as an
accumulating bf16 identity matmul (head 0) / on the DVE (head 1,
KRN_BIAS_PE=h0 default). The AV matmuls are software-pipelined one k-tile
behind (iteration order QK(kt) -> bias(kt) -> AV(kt-1)), which buries the
~1.1 us exp latency under PE work so the strict PE FIFO never stalls on
ACT, while the dense MM stream holds the HAM clock gate at 2.4 GHz.
"""

import os
import sys
import threading
from contextlib import ExitStack

import numpy as np
import ml_dtypes

_REPO = "/opt/trn_rl_repo"
if _REPO not in sys.path and os.path.isdir(_REPO):
    sys.path.insert(0, _REPO)

import concourse.bass as bass  # noqa: E402
import concourse.mybir as mybir  # noqa: E402
import concourse.tile as tile  # noqa: E402
from concourse import bacc  # noqa: E402
from concourse.bass_utils import run_bass_kernel_spmd  # noqa: E402

F32 = mybir.dt.float32
F32R = mybir.dt.float32r
BF16 = mybir.dt.bfloat16
BF16NP = ml_dtypes.bfloat16

B, SEQ, CQ = 2, 2048, 256
H, DH = 8, 64
HD = H * DH  # 512
N_CORES = 8
HPC = 2  # heads per core

# which heads' bias adds ride the PE as identity matmuls ("all"/"h0"/"none")
BIAS_PE = os.environ.get("KRN_BIAS_PE", "h0")


def _bias_on_pe(h):
    if BIAS_PE == "all":
        return True
    if BIAS_PE == "h0":
        return h == 0
    return False


def build_nc():
    nc = bacc.Bacc("TRN2", target_bir_lowering=False, debug=False)

    qxT = nc.dram_tensor("qxT", [CQ, SEQ], F32R, kind="ExternalInput").ap()
    kvxT = nc.dram_tensor("kvxT", [CQ, SEQ], F32R, kind="ExternalInput").ap()
    # host-packed: [qb, kt, h*128+p, q] so one DMA per (qb, kt) carries both
    # heads' [128, 1024] bias block (half the DMA-issue traffic)
    biasT = nc.dram_tensor("biasT", [SEQ // 1024, SEQ // 128, HPC * 128, 1024],
                           BF16, kind="ExternalInput").ap()
    wq = nc.dram_tensor("wq", [CQ, HPC * DH], F32R, kind="ExternalInput").ap()
    wk = nc.dram_tensor("wk", [CQ, HPC * DH], F32R, kind="ExternalInput").ap()
    wv = nc.dram_tensor("wv", [CQ, HPC * DH], F32, kind="ExternalInput").ap()
    wg = nc.dram_tensor("wg", [CQ, HPC * DH], F32R, kind="ExternalInput").ap()
    bg = nc.dram_tensor("bg", [HPC * DH, 1], F32, kind="ExternalInput").ap()
    wo = nc.dram_tensor("wo", [HPC * DH, CQ], F32R, kind="ExternalInput").ap()
    ident = nc.dram_tensor("ident", [128, 128], BF16, kind="ExternalInput").ap()
    outs_d = [nc.dram_tensor(f"out{h}", [SEQ, CQ], F32, kind="ExternalOutput").ap()
              for h in range(HPC)]
    rs_d = nc.dram_tensor("rs", [1, HPC, SEQ], F32, kind="ExternalOutput").ap()

    NKT = SEQ // 128  # 16 k-tiles
    P = 128
    QB = 1024
    NQB = SEQ // QB
    NTT = SEQ // P  # 16 output-projection chunks

    with tile.TileContext(nc) as tc:
        with ExitStack() as ctx:
            singles = ctx.enter_context(tc.tile_pool(name="singles", bufs=1))

            # ---- resident SBUF tensors ----
            # issue order matters: the K/Q projection inputs gate the first
            # matmuls, so their chunks go out right after their weights;
            # ident/wo/bg are not needed until much later
            w_sbs = {}
            for name, src, dt in (("wk", wk, F32R), ("wq", wq, F32R)):
                t = singles.tile([P, 2, P], dt, tag=f"w_{name}")
                (nc.sync if name == "wk" else nc.scalar).dma_start(
                    t, src.rearrange("(a p) c -> p a c", p=P))
                w_sbs[name] = t
            qxT_sb = singles.tile([P, 2, SEQ], F32R)
            kvxT_sb = singles.tile([P, 2, SEQ], F32R)

            def _in_chunk(tt):
                for a in range(2):
                    nc.sync.dma_start(
                        kvxT_sb[:, a, bass.ts(tt, 512)],
                        kvxT[a * P:(a + 1) * P, bass.ts(tt, 512)])
                    nc.scalar.dma_start(
                        qxT_sb[:, a, bass.ts(tt, 512)],
                        qxT[a * P:(a + 1) * P, bass.ts(tt, 512)])

            _in_chunk(0)
            for name, src, dt in (("wv", wv, F32), ("wg", wg, F32R)):
                t = singles.tile([P, 2, P], dt, tag=f"w_{name}")
                (nc.sync if name == "wv" else nc.scalar).dma_start(
                    t, src.rearrange("(a p) c -> p a c", p=P))
                w_sbs[name] = t
            _in_chunk(1)
            ident_sb = singles.tile([P, P], BF16)
            nc.scalar.dma_start(ident_sb, ident)
            _in_chunk(2)
            _in_chunk(3)
            bg_sb = singles.tile([P, 1], F32)
            nc.sync.dma_start(bg_sb, bg)
            wo_sb = singles.tile([P, CQ], F32R)  # heads stacked on partitions
            nc.scalar.dma_start(wo_sb, wo)

            KT_sb = singles.tile([P, SEQ], F32R)   # [2h x 64 d, k]
            QT_sb = singles.tile([P, SEQ], F32R)   # [2h x 64 d, q]
            GT_sb = singles.tile([P, SEQ], F32)    # gate, [2 heads x 64, q]
            V_sb = singles.tile([P, HPC, NKT, DH + 1], F32R)  # [k%128, h, kt, d|1]
            OG_sb = singles.tile([P, SEQ], F32R)   # (o * g)^T, heads stacked
            rs_sb = singles.tile([1, HPC, SEQ], F32)   # softmax denominators
            ones_col = V_sb[:, :, :, DH:DH + 1].bitcast(F32)
            nc.vector.memset(ones_col, 1.0)

            # ---- single shared PSUM layout: OT pool (4 banks) + S pool
            # (4 banks); projection and output-projection tiles ride these
            # pools, so there is no pool-close barrier anywhere ----
            with tc.tile_pool(name="otpsum", bufs=2, space="PSUM") as otpool, \
                 tc.tile_pool(name="spsum", bufs=2, space="PSUM") as spool, \
                 tc.tile_pool(name="biasp", bufs=10) as biaspool, \
                 tc.tile_pool(name="sbp", bufs=4) as sbpool, \
                 tc.tile_pool(name="ostg", bufs=2) as ostgpool, \
                 tc.tile_pool(name="ep", bufs=6) as epool:

                # ---- stage B: projections (f32r; V's is fp32 because its
                # moving dim is only 128 where f32r runs 1/4 rate anyway) ----
                def proj_kq(wt, x_sb, dst, tt):
                    ps = spool.tile([P, 512], F32, tag="s", name="proj")
                    nc.tensor.matmul(ps, wt[:, 0, :],
                                     x_sb[:, 0, bass.ts(tt, 512)],
                                     start=True, stop=False)
                    nc.tensor.matmul(ps, wt[:, 1, :],
                                     x_sb[:, 1, bass.ts(tt, 512)],
                                     start=False, stop=True)
                    nc.vector.tensor_copy(dst[:, bass.ts(tt, 512)], ps)

                for tt in range(4):
                    proj_kq(w_sbs["wk"], kvxT_sb, KT_sb, tt)
                    proj_kq(w_sbs["wq"], qxT_sb, QT_sb, tt)
                for kt in range(NKT):
                    ps = spool.tile([P, P], F32, tag="s", name="vproj")
                    nc.tensor.matmul(ps,
                                     kvxT_sb[:, 0, bass.ts(kt, P)].bitcast(F32),
                                     w_sbs["wv"][:, 0, :],
                                     start=True, stop=False)
                    nc.tensor.matmul(ps,
                                     kvxT_sb[:, 1, bass.ts(kt, P)].bitcast(F32),
                                     w_sbs["wv"][:, 1, :],
                                     start=False, stop=True)
                    nc.vector.tensor_copy(V_sb[:, 0, kt, 0:DH], ps[:, 0:DH])
                    nc.vector.tensor_copy(V_sb[:, 1, kt, 0:DH], ps[:, DH:2 * DH])
                for tt in range(4):
                    ps = spool.tile([P, 512], F32, tag="s", name="projg")
                    nc.tensor.matmul(ps, w_sbs["wg"][:, 0, :],
                                     qxT_sb[:, 0, bass.ts(tt, 512)],
                                     start=True, stop=False)
                    nc.tensor.matmul(ps, w_sbs["wg"][:, 1, :],
                                     qxT_sb[:, 1, bass.ts(tt, 512)],
                                     start=False, stop=True)
                    nc.scalar.activation(GT_sb[:, bass.ts(tt, 512)], ps,
                                         mybir.ActivationFunctionType.Sigmoid,
                                         bias=bg_sb)

                # output projections for one q-block half (8 token chunks x
                # 2 heads). Both heads' [64,128] lhsT sit on partitions
                # 0-63 / 64-127 -> row groups (0,0)/(64,0), so each pair
                # runs concurrently in the PE array. PSUM rides the OT and
                # (in the tail) S pools; results stage into one SBUF tile
                # per head and leave as a single 1 MB strided DMA, so the
                # sequencers see 2 DMA issues instead of 16.
                def fin_half(qb, pools=(otpool,), copy_engs=(nc.vector,)):
                    stg = [ostgpool.tile([P, NTT // 2, CQ], F32, tag="OSTG",
                                         name=f"ostg{qb}_{h}")
                           for h in range(HPC)]
                    for ti, tt in enumerate(range(qb * NTT // 2,
                                                  (qb + 1) * NTT // 2)):
                        pss = []
                        for h in range(HPC):
                            ps = pools[(tt + h) % len(pools)].tile(
                                [P, CQ], F32,
                                tag="ot"
                                if pools[(tt + h) % len(pools)] is otpool
                                else "s",
                                name=f"fin{h}_{tt}")
                            nc.tensor.matmul(ps,
                                             OG_sb[h * DH:(h + 1) * DH,
                                                   bass.ts(tt, P)],
                                             wo_sb[h * DH:(h + 1) * DH, :],
                                             start=True, stop=True)
                            pss.append(ps)
                        for h in range(HPC):
                            ceng = copy_engs[(tt + h) % len(copy_engs)]
                            if ceng is nc.scalar:
                                ceng.copy(stg[h][:, ti, :], pss[h])
                            else:
                                ceng.tensor_copy(stg[h][:, ti, :], pss[h])
                    for h in range(HPC):
                        dst = outs_d[h][qb * (SEQ // 2):(qb + 1) * (SEQ // 2),
                                        :]
                        (nc.sync if h == 0 else nc.scalar).dma_start(
                            dst.rearrange("(t p) c -> p t c", p=P), stg[h])

                # ---- stage C: attention, q-block outer, AV software-
                # pipelined one k-tile behind ----
                ndma = 0
                for qb in range(NQB):
                    q0 = qb * QB
                    OTs = [otpool.tile([DH + 1, QB], F32, name=f"OT{qb}_{h}",
                                       tag="ot") for h in range(HPC)]
                    prev = None  # (Ss_prev consumed; Es of kt-1)

                    def av(kt, Es):
                        for h in range(HPC):
                            for j in range(2):
                                nc.tensor.matmul(
                                    OTs[h][:, bass.ts(j, 512)],
                                    V_sb[:, h, kt, :],
                                    Es[h][:, bass.ts(j, 512)],
                                    start=(kt == 0), stop=(kt == NKT - 1))

                    for kt in range(NKT):
                        # one DMA per (qb, kt) carries both heads; only the
                        # sync/gpsimd queues touch it so the scalar
                        # sequencer stays clean for exp dispatch
                        bias_sb = biaspool.tile([P, HPC, QB], BF16)
                        dma_eng = (nc.gpsimd, nc.sync)[ndma % 2]
                        ndma += 1
                        dma_eng.dma_start(
                            bias_sb,
                            biasT[qb, kt].rearrange("(h p) q -> p h q", p=P))
                        bias_t = [bias_sb[:, h, :] for h in range(HPC)]
                        Ss = [spool.tile([P, QB], F32, tag="s",
                                         name=f"S{qb}_{kt}_{h}")
                              for h in range(HPC)]
                        # packed QK: adjacent instructions on disjoint row
                        # groups execute concurrently in the array
                        for j in range(2):
                            for h in range(HPC):
                                hsl = slice(h * DH, (h + 1) * DH)
                                nc.tensor.matmul(
                                    Ss[h][:, bass.ts(j, 512)],
                                    KT_sb[hsl, bass.ts(kt, P)],
                                    QT_sb[hsl, bass.ds(q0 + j * 512, 512)],
                                    start=True, stop=not _bias_on_pe(h))
                        # bias adds + exp for this kt
                        Es = []
                        for h in range(HPC):
                            E = epool.tile([P, QB], F32R)
                            if _bias_on_pe(h):
                                for j in range(2):
                                    nc.tensor.matmul(
                                        Ss[h][:, bass.ts(j, 512)],
                                        ident_sb,
                                        bias_t[h][:, bass.ts(j, 512)],
                                        start=False, stop=True)
                                nc.scalar.activation(
                                    E, Ss[h], mybir.ActivationFunctionType.Exp)
                            else:
                                # add in 512-col halves: the first half can
                                # start while the second QK matmul streams,
                                # and the S slot frees earlier (more chances
                                # for the next k-tile's QK pair to dispatch
                                # together and overlap in the array)
                                SB = sbpool.tile([P, QB], F32, tag="SB")
                                for j in range(2):
                                    nc.vector.tensor_add(
                                        SB[:, bass.ts(j, 512)],
                                        Ss[h][:, bass.ts(j, 512)],
                                        bias_t[h][:, bass.ts(j, 512)])
                                nc.scalar.activation(
                                    E, SB, mybir.ActivationFunctionType.Exp)
                            Es.append(E)
                        # AV for the PREVIOUS k-tile: its exps finished long
                        # ago, so the PE never waits on ACT here
                        if prev is not None:
                            av(kt - 1, prev)
                        prev = Es
                    av(NKT - 1, prev)  # drain

                    # epilogue for this q-block, both heads
                    for h in range(HPC):
                        hsl = slice(h * DH, (h + 1) * DH)
                        OT = OTs[h]
                        if qb == NQB - 1 and h == HPC - 1:
                            nc.scalar.copy(rs_sb[:, h, bass.ds(q0, QB)],
                                           OT[DH:DH + 1, :])
                        else:
                            nc.vector.tensor_copy(rs_sb[:, h, bass.ds(q0, QB)],
                                                  OT[DH:DH + 1, :])
                        nc.vector.tensor_mul(OG_sb[hsl, bass.ds(q0, QB)],
                                             GT_sb[hsl, bass.ds(q0, QB)],
                                             OT[0:DH, :])
                    # output projections for the finished q-block slot into
                    # the OT pool while it is free (qb 0: during the qb
                    # boundary, copies on the DVE; qb 1: the tail, where the
                    # S pool and ACT are also free -> 4 psum slots and two
                    # copy engines)
                    if qb == 0:
                        fin_half(0)
                    else:
                        fin_half(1, pools=(otpool, spool),
                                 copy_engs=(nc.vector, nc.scalar))

            nc.sync.dma_start(rs_d, rs_sb)

    nc.compile()
    return nc


_NC = None
_NC_LOCK = threading.Lock()


def _get_nc():
    global _NC
    with _NC_LOCK:
        if _NC is None:
            _NC = build_nc()
        return _NC


def make_in_maps(q_x, kv_x, bias, w_q, w_k, w_v, w_g, b_g, w_o, b_o):
    del b_o  # added on the host after the gather
    q_x = np.asarray(q_x, dtype=np.float32)
    kv_x = np.asarray(kv_x, dtype=np.float32)
    bias = np.asarray(bias, dtype=np.float32)
    w_q = np.asarray(w_q, dtype=np.float32) * np.float32(0.125)  # fold 1/sqrt(64)
    w_k = np.asarray(w_k, dtype=np.float32)
    w_v = np.asarray(w_v, dtype=np.float32)
    w_g = np.asarray(w_g, dtype=np.float32)
    b_g = np.asarray(b_g, dtype=np.float32)
    w_o = np.asarray(w_o, dtype=np.float32)
    ident = np.eye(128, dtype=BF16NP)

    in_maps = []
    for c in range(N_CORES):
        b = c // (N_CORES // B)
        h0 = HPC * (c % (N_CORES // B))
        cols = slice(h0 * DH, (h0 + HPC) * DH)
        in_maps.append({
            "qxT": np.ascontiguousarray(q_x[b].T),
            "kvxT": np.ascontiguousarray(kv_x[b].T),
            # [h, k, q] -> [qb, kt, h*128+p, q] so one DMA per (qb, kt)
            # loads both heads' bias block
            "biasT": np.ascontiguousarray(
                bias[b, h0:h0 + HPC].swapaxes(1, 2)
                .reshape(HPC, SEQ // 128, 128, SEQ // 1024, 1024)
                .transpose(3, 1, 0, 2, 4)
                .reshape(SEQ // 1024, SEQ // 128, HPC * 128, 1024)
                .astype(BF16NP)),
            "wq": np.ascontiguousarray(w_q[:, cols]),
            "wk": np.ascontiguousarray(w_k[:, cols]),
            "wv": np.ascontiguousarray(w_v[:, cols]),
            "wg": np.ascontiguousarray(w_g[:, cols]),
            "bg": np.ascontiguousarray(b_g[cols].reshape(HPC * DH, 1)),
            "wo": np.ascontiguousarray(w_o[cols, :]),
            "ident": ident,
        })
    return in_maps


def gather_output(results, b_o):
    full = np.zeros((B, SEQ, CQ), dtype=np.float32)
    for c in range(N_CORES):
        b = c // (N_CORES // B)
        rs = results[c]["rs"][0]
        for h in range(HPC):
            full[b] += results[c][f"out{h}"] / rs[h][:, None]
    full += np.asarray(b_o, dtype=np.float32)
    return full


def kernel(**inputs):
    nc = _get_nc()
    in_maps = make_in_maps(**inputs)
    res = run_bass_kernel_spmd(nc, in_maps, core_ids=list(range(N_CORES)))
    return gather_output(res.results, inputs["b_o"])


# revision 27
# speedup vs baseline: 1.2815x; 1.2815x over previous
"""Trainium2 Bass kernel for nn_Attention_3934190044008.

Multi-head attention with additive bias and sigmoid gating:
  q = (q_x @ w_q) / 8, k = kv_x @ w_k, v = kv_x @ w_v   (8 heads x 64)
  a = softmax(q k^T + bias);  o = a @ v
  o = o * sigmoid(q_x @ w_g + b_g);  out = o @ w_o + b_o

Sharding: 16 (batch, head) pairs over 8 cores -> each core owns one batch
element and 2 heads, produces a partial [2048, 256] output contribution
(o_slice @ w_o rows); host sums the 4 partials per batch and adds b_o.

Device-side layout is "feature on partitions" (transposed): scores are
computed as S^T [k, q] so the softmax denominator rides the AV matmul via a
ones-column appended to V, and softmax-over-k never needs a partition-axis
reduction. All transposes are done on the host (numpy).

v5: q-block-outer loop, both heads interleaved per k-tile. The two heads'
64-contract QK matmuls sit on partitions 0-63 / 64-127 so they land in
disjoint PE row groups and run CONCURRENTLY (hw-verified 2x: 114 ns/MM
packed vs 227 serial); same packing for the per-head output projections.
All matmuls are f32r (tf32-class, 1 cycle/row); the additive bias ships
as bf16 (halves the 33.5 MB/core stream) and is added on the PE as an
accumulating bf16 identity matmul (head 0) / on the DVE (head 1,
KRN_BIAS_PE=h0 default). The AV matmuls are software-pipelined one k-tile
behind (iteration order QK(kt) -> bias(kt) -> AV(kt-1)), which buries the
~1.1 us exp latency under PE work so the strict PE FIFO never stalls on
ACT, while the dense MM stream holds the HAM clock gate at 2.4 GHz.
"""

import os
import sys
import threading
from contextlib import ExitStack

import numpy as np
import ml_dtypes

_REPO = "/opt/trn_rl_repo"
if _REPO not in sys.path and os.path.isdir(_REPO):
    sys.path.insert(0, _REPO)

import concourse.bass as bass  # noqa: E402
import concourse.mybir as mybir  # noqa: E402
import concourse.tile as tile  # noqa: E402
from concourse import bacc  # noqa: E402
from concourse.bass_utils import run_bass_kernel_spmd  # noqa: E402

F32 = mybir.dt.float32
F32R = mybir.dt.float32r
BF16 = mybir.dt.bfloat16
BF16NP = ml_dtypes.bfloat16

B, SEQ, CQ = 2, 2048, 256
H, DH = 8, 64
HD = H * DH  # 512
N_CORES = 8
HPC = 2  # heads per core

# which heads' bias adds ride the PE as identity matmuls ("all"/"h0"/"none")
BIAS_PE = os.environ.get("KRN_BIAS_PE", "h0")


def _bias_on_pe(h):
    if BIAS_PE == "all":
        return True
    if BIAS_PE == "h0":
        return h == 0
    return False


def build_nc():
    nc = bacc.Bacc("TRN2", target_bir_lowering=False, debug=False)

    qxT = nc.dram_tensor("qxT", [CQ, SEQ], F32R, kind="ExternalInput").ap()
    kvxT = nc.dram_tensor("kvxT", [CQ, SEQ], F32R, kind="ExternalInput").ap()
    # host-packed: [qb, kt, h*128+p, q] so one DMA per (qb, kt) carries both
    # heads' [128, 1024] bias block (half the DMA-issue traffic)
    biasT = nc.dram_tensor("biasT", [SEQ // 1024, SEQ // 128, HPC * 128, 1024],
                           BF16, kind="ExternalInput").ap()
    wq = nc.dram_tensor("wq", [CQ, HPC * DH], F32R, kind="ExternalInput").ap()
    wk = nc.dram_tensor("wk", [CQ, HPC * DH], F32R, kind="ExternalInput").ap()
    wv = nc.dram_tensor("wv", [CQ, HPC * DH], F32, kind="ExternalInput").ap()
    wg = nc.dram_tensor("wg", [CQ, HPC * DH], F32R, kind="ExternalInput").ap()
    bg = nc.dram_tensor("bg", [HPC * DH, 1], F32, kind="ExternalInput").ap()
    wo = nc.dram_tensor("wo", [HPC * DH, CQ], F32R, kind="ExternalInput").ap()
    ident = nc.dram_tensor("ident", [128, 128], BF16, kind="ExternalInput").ap()
    outs_d = [nc.dram_tensor(f"out{h}", [SEQ, CQ], F32, kind="ExternalOutput").ap()
              for h in range(HPC)]
    rs_d = nc.dram_tensor("rs", [1, HPC, SEQ], F32, kind="ExternalOutput").ap()

    NKT = SEQ // 128  # 16 k-tiles
    P = 128
    QB = 1024
    NQB = SEQ // QB
    NTT = SEQ // P  # 16 output-projection chunks

    with tile.TileContext(nc) as tc:
        with ExitStack() as ctx:
            singles = ctx.enter_context(tc.tile_pool(name="singles", bufs=1))

            # ---- resident SBUF tensors ----
            # issue order matters: the K/Q projection inputs gate the first
            # matmuls, so their chunks go out right after their weights;
            # ident/wo/bg are not needed until much later
            w_sbs = {}
            for name, src, dt in (("wk", wk, F32R), ("wq", wq, F32R)):
                t = singles.tile([P, 2, P], dt, tag=f"w_{name}")
                (nc.sync if name == "wk" else nc.scalar).dma_start(
                    t, src.rearrange("(a p) c -> p a c", p=P))
                w_sbs[name] = t
            qxT_sb = singles.tile([P, 2, SEQ], F32R)
            kvxT_sb = singles.tile([P, 2, SEQ], F32R)

            def _in_chunk(tt):
                for a in range(2):
                    nc.sync.dma_start(
                        kvxT_sb[:, a, bass.ts(tt, 512)],
                        kvxT[a * P:(a + 1) * P, bass.ts(tt, 512)])
                    nc.scalar.dma_start(
                        qxT_sb[:, a, bass.ts(tt, 512)],
                        qxT[a * P:(a + 1) * P, bass.ts(tt, 512)])

            _in_chunk(0)
            for name, src, dt in (("wv", wv, F32), ("wg", wg, F32R)):
                t = singles.tile([P, 2, P], dt, tag=f"w_{name}")
                (nc.sync if name == "wv" else nc.scalar).dma_start(
                    t, src.rearrange("(a p) c -> p a c", p=P))
                w_sbs[name] = t
            _in_chunk(1)
            ident_sb = singles.tile([P, P], BF16)
            nc.scalar.dma_start(ident_sb, ident)
            _in_chunk(2)
            _in_chunk(3)
            bg_sb = singles.tile([P, 1], F32)
            nc.sync.dma_start(bg_sb, bg)
            wo_sb = singles.tile([P, CQ], F32R)  # heads stacked on partitions
            nc.scalar.dma_start(wo_sb, wo)

            KT_sb = singles.tile([P, SEQ], F32R)   # [2h x 64 d, k]
            QT_sb = singles.tile([P, SEQ], F32R)   # [2h x 64 d, q]
            GT_sb = singles.tile([P, SEQ], F32)    # gate, [2 heads x 64, q]
            V_sb = singles.tile([P, HPC, NKT, DH + 1], F32R)  # [k%128, h, kt, d|1]
            OG_sb = singles.tile([P, SEQ], F32R)   # (o * g)^T, heads stacked
            rs_sb = singles.tile([1, HPC, SEQ], F32)   # softmax denominators
            ones_col = V_sb[:, :, :, DH:DH + 1].bitcast(F32)
            nc.vector.memset(ones_col, 1.0)

            # ---- single shared PSUM layout: OT pool (4 banks) + S pool
            # (4 banks); projection and output-projection tiles ride these
            # pools, so there is no pool-close barrier anywhere ----
            with tc.tile_pool(name="otpsum", bufs=2, space="PSUM") as otpool, \
                 tc.tile_pool(name="spsum", bufs=2, space="PSUM") as spool, \
                 tc.tile_pool(name="biasp", bufs=10) as biaspool, \
                 tc.tile_pool(name="sbp", bufs=4) as sbpool, \
                 tc.tile_pool(name="ostg", bufs=2) as ostgpool, \
                 tc.tile_pool(name="ep", bufs=6) as epool:

                # ---- stage B: projections (f32r; V's is fp32 because its
                # moving dim is only 128 where f32r runs 1/4 rate anyway) ----
                def proj_kq(wt, x_sb, dst, tt):
                    ps = spool.tile([P, 512], F32, tag="s", name="proj")
                    nc.tensor.matmul(ps, wt[:, 0, :],
                                     x_sb[:, 0, bass.ts(tt, 512)],
                                     start=True, stop=False)
                    nc.tensor.matmul(ps, wt[:, 1, :],
                                     x_sb[:, 1, bass.ts(tt, 512)],
                                     start=False, stop=True)
                    nc.vector.tensor_copy(dst[:, bass.ts(tt, 512)], ps)

                def proj_v(kt):
                    ps = spool.tile([P, P], F32, tag="s", name="vproj")
                    nc.tensor.matmul(ps,
                                     kvxT_sb[:, 0, bass.ts(kt, P)].bitcast(F32),
                                     w_sbs["wv"][:, 0, :],
                                     start=True, stop=False)
                    nc.tensor.matmul(ps,
                                     kvxT_sb[:, 1, bass.ts(kt, P)].bitcast(F32),
                                     w_sbs["wv"][:, 1, :],
                                     start=False, stop=True)
                    nc.vector.tensor_copy(V_sb[:, 0, kt, 0:DH], ps[:, 0:DH])
                    nc.vector.tensor_copy(V_sb[:, 1, kt, 0:DH], ps[:, DH:2 * DH])

                # V projections interleaved per input chunk: when a kvxT
                # chunk lands there is always ready PE work even while the
                # matching qxT chunk is still in flight (denser startup
                # stream -> fewer HAM re-throttles)
                for tt in range(4):
                    proj_kq(w_sbs["wk"], kvxT_sb, KT_sb, tt)
                    proj_kq(w_sbs["wq"], qxT_sb, QT_sb, tt)
                    for kt in range(4 * tt, 4 * tt + 4):
                        proj_v(kt)
                for tt in range(4):
                    ps = spool.tile([P, 512], F32, tag="s", name="projg")
                    nc.tensor.matmul(ps, w_sbs["wg"][:, 0, :],
                                     qxT_sb[:, 0, bass.ts(tt, 512)],
                                     start=True, stop=False)
                    nc.tensor.matmul(ps, w_sbs["wg"][:, 1, :],
                                     qxT_sb[:, 1, bass.ts(tt, 512)],
                                     start=False, stop=True)
                    nc.scalar.activation(GT_sb[:, bass.ts(tt, 512)], ps,
                                         mybir.ActivationFunctionType.Sigmoid,
                                         bias=bg_sb)

                # output projections for one q-block half (8 token chunks x
                # 2 heads). Both heads' [64,128] lhsT sit on partitions
                # 0-63 / 64-127 -> row groups (0,0)/(64,0), so each pair
                # runs concurrently in the PE array. PSUM rides the OT and
                # (in the tail) S pools; results stage into one SBUF tile
                # per head and leave as a single 1 MB strided DMA, so the
                # sequencers see 2 DMA issues instead of 16.
                def fin_half(qb, pools=(otpool,), copy_engs=(nc.vector,)):
                    stg = [ostgpool.tile([P, NTT // 2, CQ], F32, tag="OSTG",
                                         name=f"ostg{qb}_{h}")
                           for h in range(HPC)]
                    for ti, tt in enumerate(range(qb * NTT // 2,
                                                  (qb + 1) * NTT // 2)):
                        pss = []
                        for h in range(HPC):
                            ps = pools[(tt + h) % len(pools)].tile(
                                [P, CQ], F32,
                                tag="ot"
                                if pools[(tt + h) % len(pools)] is otpool
                                else "s",
                                name=f"fin{h}_{tt}")
                            nc.tensor.matmul(ps,
                                             OG_sb[h * DH:(h + 1) * DH,
                                                   bass.ts(tt, P)],
                                             wo_sb[h * DH:(h + 1) * DH, :],
                                             start=True, stop=True)
                            pss.append(ps)
                        for h in range(HPC):
                            ceng = copy_engs[(tt + h) % len(copy_engs)]
                            if ceng is nc.scalar:
                                ceng.copy(stg[h][:, ti, :], pss[h])
                            else:
                                ceng.tensor_copy(stg[h][:, ti, :], pss[h])
                    # ship each head's half in two DMAs so the last bytes
                    # leave right after the last copy instead of waiting for
                    # the whole staging tile
                    for h in range(HPC):
                        for piece in range(2):
                            r0 = qb * (SEQ // 2) + piece * (SEQ // 4)
                            dst = outs_d[h][r0:r0 + SEQ // 4, :]
                            (nc.sync if h == 0 else nc.scalar).dma_start(
                                dst.rearrange("(t p) c -> p t c", p=P),
                                stg[h][:, piece * (NTT // 4):
                                       (piece + 1) * (NTT // 4), :])

                # ---- stage C: attention, q-block outer, AV software-
                # pipelined one k-tile behind ----
                ndma = 0
                for qb in range(NQB):
                    q0 = qb * QB
                    OTs = [otpool.tile([DH + 1, QB], F32, name=f"OT{qb}_{h}",
                                       tag="ot") for h in range(HPC)]
                    prev = None  # (Ss_prev consumed; Es of kt-1)

                    def av(kt, Es):
                        for h in range(HPC):
                            for j in range(2):
                                nc.tensor.matmul(
                                    OTs[h][:, bass.ts(j, 512)],
                                    V_sb[:, h, kt, :],
                                    Es[h][:, bass.ts(j, 512)],
                                    start=(kt == 0), stop=(kt == NKT - 1))

                    for kt in range(NKT):
                        # one DMA per (qb, kt) carries both heads; only the
                        # sync/gpsimd queues touch it so the scalar
                        # sequencer stays clean for exp dispatch
                        bias_sb = biaspool.tile([P, HPC, QB], BF16)
                        dma_eng = (nc.gpsimd, nc.sync)[ndma % 2]
                        ndma += 1
                        dma_eng.dma_start(
                            bias_sb,
                            biasT[qb, kt].rearrange("(h p) q -> p h q", p=P))
                        bias_t = [bias_sb[:, h, :] for h in range(HPC)]
                        Ss = [spool.tile([P, QB], F32, tag="s",
                                         name=f"S{qb}_{kt}_{h}")
                              for h in range(HPC)]
                        # bias FIRST for the PE-path heads: the identity
                        # matmul opens the accumulation group, so the QK
                        # matmuls close it and exp can start right after
                        # them (instead of after trailing bias matmuls).
                        # Both heads' S slots then free at about the same
                        # time, which lets the next k-tile's QK pair
                        # dispatch back-to-back and overlap in the array.
                        for h in range(HPC):
                            if _bias_on_pe(h):
                                for j in range(2):
                                    nc.tensor.matmul(
                                        Ss[h][:, bass.ts(j, 512)],
                                        ident_sb,
                                        bias_t[h][:, bass.ts(j, 512)],
                                        start=True, stop=False)
                        # packed QK: adjacent instructions on disjoint row
                        # groups execute concurrently in the array
                        for j in range(2):
                            for h in range(HPC):
                                hsl = slice(h * DH, (h + 1) * DH)
                                nc.tensor.matmul(
                                    Ss[h][:, bass.ts(j, 512)],
                                    KT_sb[hsl, bass.ts(kt, P)],
                                    QT_sb[hsl, bass.ds(q0 + j * 512, 512)],
                                    start=not _bias_on_pe(h), stop=True)
                        # exp / DVE-add for this kt
                        Es = []
                        for h in range(HPC):
                            E = epool.tile([P, QB], F32R)
                            if _bias_on_pe(h):
                                nc.scalar.activation(
                                    E, Ss[h], mybir.ActivationFunctionType.Exp)
                            else:
                                # add in 512-col halves: the first half can
                                # start while the second QK matmul streams,
                                # and the S slot frees earlier (more chances
                                # for the next k-tile's QK pair to dispatch
                                # together and overlap in the array)
                                SB = sbpool.tile([P, QB], F32, tag="SB")
                                for j in range(2):
                                    nc.vector.tensor_add(
                                        SB[:, bass.ts(j, 512)],
                                        Ss[h][:, bass.ts(j, 512)],
                                        bias_t[h][:, bass.ts(j, 512)])
                                nc.scalar.activation(
                                    E, SB, mybir.ActivationFunctionType.Exp)
                            Es.append(E)
                        # AV for the PREVIOUS k-tile: its exps finished long
                        # ago, so the PE never waits on ACT here
                        if prev is not None:
                            av(kt - 1, prev)
                        prev = Es
                    av(NKT - 1, prev)  # drain

                    # epilogue for this q-block, both heads
                    for h in range(HPC):
                        hsl = slice(h * DH, (h + 1) * DH)
                        OT = OTs[h]
                        if qb == NQB - 1 and h == HPC - 1:
                            nc.scalar.copy(rs_sb[:, h, bass.ds(q0, QB)],
                                           OT[DH:DH + 1, :])
                        else:
                            nc.vector.tensor_copy(rs_sb[:, h, bass.ds(q0, QB)],
                                                  OT[DH:DH + 1, :])
                        nc.vector.tensor_mul(OG_sb[hsl, bass.ds(q0, QB)],
                                             GT_sb[hsl, bass.ds(q0, QB)],
                                             OT[0:DH, :])
                    # output projections for the finished q-block slot into
                    # the OT pool while it is free (qb 0: during the qb
                    # boundary, copies on the DVE; qb 1: the tail, where the
                    # S pool and ACT are also free -> 4 psum slots and two
                    # copy engines)
                    if qb == 0:
                        fin_half(0)
                    else:
                        # rs is complete here; ship it before the tail fins
                        # so it overlaps them instead of trailing the kernel
                        nc.gpsimd.dma_start(rs_d, rs_sb)
                        fin_half(1, pools=(otpool, spool),
                                 copy_engs=(nc.vector, nc.scalar))

    nc.compile()
    return nc


_NC = None
_NC_LOCK = threading.Lock()


def _get_nc():
    global _NC
    with _NC_LOCK:
        if _NC is None:
            _NC = build_nc()
        return _NC


def make_in_maps(q_x, kv_x, bias, w_q, w_k, w_v, w_g, b_g, w_o, b_o):
    del b_o  # added on the host after the gather
    q_x = np.asarray(q_x, dtype=np.float32)
    kv_x = np.asarray(kv_x, dtype=np.float32)
    bias = np.asarray(bias, dtype=np.float32)
    w_q = np.asarray(w_q, dtype=np.float32) * np.float32(0.125)  # fold 1/sqrt(64)
    w_k = np.asarray(w_k, dtype=np.float32)
    w_v = np.asarray(w_v, dtype=np.float32)
    w_g = np.asarray(w_g, dtype=np.float32)
    b_g = np.asarray(b_g, dtype=np.float32)
    w_o = np.asarray(w_o, dtype=np.float32)
    ident = np.eye(128, dtype=BF16NP)

    in_maps = []
    for c in range(N_CORES):
        b = c // (N_CORES // B)
        h0 = HPC * (c % (N_CORES // B))
        cols = slice(h0 * DH, (h0 + HPC) * DH)
        in_maps.append({
            "qxT": np.ascontiguousarray(q_x[b].T),
            "kvxT": np.ascontiguousarray(kv_x[b].T),
            # [h, k, q] -> [qb, kt, h*128+p, q] so one DMA per (qb, kt)
            # loads both heads' bias block
            "biasT": np.ascontiguousarray(
                bias[b, h0:h0 + HPC].swapaxes(1, 2)
                .reshape(HPC, SEQ // 128, 128, SEQ // 1024, 1024)
                .transpose(3, 1, 0, 2, 4)
                .reshape(SEQ // 1024, SEQ // 128, HPC * 128, 1024)
                .astype(BF16NP)),
            "wq": np.ascontiguousarray(w_q[:, cols]),
            "wk": np.ascontiguousarray(w_k[:, cols]),
            "wv": np.ascontiguousarray(w_v[:, cols]),
            "wg": np.ascontiguousarray(w_g[:, cols]),
            "bg": np.ascontiguousarray(b_g[cols].reshape(HPC * DH, 1)),
            "wo": np.ascontiguousarray(w_o[cols, :]),
            "ident": ident,
        })
    return in_maps


def gather_output(results, b_o):
    full = np.zeros((B, SEQ, CQ), dtype=np.float32)
    for c in range(N_CORES):
        b = c // (N_CORES // B)
        rs = results[c]["rs"][0]
        for h in range(HPC):
            full[b] += results[c][f"out{h}"] / rs[h][:, None]
    full += np.asarray(b_o, dtype=np.float32)
    return full


def kernel(**inputs):
    nc = _get_nc()
    in_maps = make_in_maps(**inputs)
    res = run_bass_kernel_spmd(nc, in_maps, core_ids=list(range(N_CORES)))
    return gather_output(res.results, inputs["b_o"])
